# revision 7
# baseline (speedup 1.0000x reference)
"""ArcFace loss on 8 TRN2 NeuronCores — class-parallel (vocab-sharded).

Math: loss = mean_b[ M0 + ln(Z'_b) - s*phi_b ] with
  Z_b  = sum_c exp(s*cos(b,c) - M0)          (device, sharded over classes)
  Z'_b = Z_b - exp(s*cos(b,l_b) - M0) + exp(s*phi_b - M0)   (label correction)
M0 is a fixed logsumexp shift: |s*cos| <= ~32 for unit-norm rows, so
exp(s*cos - 40) never overflows and stays in normal f32 range.

Host (inside kernel()): row-normalize x and W, fold nothing into W, transpose
W shards to [D, C_shard] so the device needs no on-chip transposes, and
evaluate the tiny label/phi terms ([512] vectors). Device: the 512x512x100k
matmul, exp, row-sums, an AllGather of per-core partial Z, and the final
scalar reduction.
"""

import math

import numpy as np

from concourse import bacc, bass, mybir
from concourse.bass_utils import run_bass_kernel_spmd
from concourse.tile import TileContext

NCORES = 8
B = 512
D = 512
C = 100000
CS = 12544  # per-core classes, padded: 8 * 12544 = 100352 >= C
S = 120.0
MARGIN = 0.3
COS_M = math.cos(MARGIN)
SIN_M = math.sin(MARGIN)
TH = math.cos(math.pi - MARGIN)
MM = math.sin(math.pi - MARGIN) * MARGIN
M0 = 40.0  # logsumexp shift
SUPER = 2048  # class columns per DMA (1 MiB per [128, 2048] f32 tile)
NBLK = 512  # class columns per matmul (one PSUM bank)

F32 = mybir.dt.float32
FN = mybir.ActivationFunctionType

_GRAPH = None
LAST_RESULT = None  # BassKernelResults of the most recent run (for test harness)


def _build_nc():
    nc = bacc.Bacc("TRN2", target_bir_lowering=False)

    # const AP for the Exp bias (only 0.0/1.0 are pre-registered)
    _cb = nc.alloc_sbuf_tensor(f"const-float32-{-M0}", [128, 1], F32)
    nc.gpsimd.memset(_cb.ap(), -M0)
    nc.const_aps.aps[(F32, -M0)] = _cb.ap()
    nc.all_engine_barrier()

    xt = nc.declare_dram_parameter("xt", [D, B], F32, isOutput=False)
    wt = nc.declare_dram_parameter("wt", [D, CS], F32, isOutput=False)
    sl = nc.declare_dram_parameter("sl", [1, B], F32, isOutput=False)
    su = nc.declare_dram_parameter("su", [1, B], F32, isOutput=False)
    out = nc.declare_dram_parameter("out", [1, 1], F32, isOutput=True)

    with TileContext(nc, num_cores=NCORES) as tc:
        with (
            tc.tile_pool(name="xpool", bufs=1) as xpool,
            tc.tile_pool(name="wpool", bufs=2) as wpool,
            tc.tile_pool(name="epool", bufs=4) as epool,
            tc.tile_pool(name="zpool", bufs=1) as zpool,
            tc.tile_pool(name="spool", bufs=1) as spool,
            tc.tile_pool(name="psum", bufs=8, space="PSUM") as pp,
            tc.tile_pool(name="dram", bufs=1, space="DRAM") as dram,
        ):
            # x^T (normalized, transposed on host): 4 contraction chunks [128, B]
            xts = []
            for k in range(4):
                t = xpool.tile([128, B], F32, tag=f"xt{k}", name=f"xts{k}")
                nc.sync.dma_start(t[:], xt[k * 128 : (k + 1) * 128, :])
                xts.append(t)

            sl_sb = spool.tile([1, B], F32, tag="sl")
            nc.sync.dma_start(sl_sb[:], sl[:])
            su_sb = spool.tile([1, B], F32, tag="su")
            nc.sync.dma_start(su_sb[:], su[:])

            # per-batch-tile accumulators of per-block exp-sums (one col/block)
            zbufs = [zpool.tile([128, 32], F32, tag=f"zb{bi}", name=f"zb{bi}") for bi in range(4)]

            col = 0
            c0 = 0
            while c0 < CS:
                sw = min(SUPER, CS - c0)
                wts = []
                for k in range(4):
                    t = wpool.tile([128, SUPER], F32, tag=f"w{k}", name=f"wts{k}")
                    nc.sync.dma_start(
                        t[:, :sw], wt[k * 128 : (k + 1) * 128, c0 : c0 + sw]
                    )
                    wts.append(t)
                nb0 = 0
                while nb0 < sw:
                    nb = min(NBLK, sw - nb0)
                    for bi in range(4):
                        ps = pp.tile([128, NBLK], F32, tag="ps")
                        for k in range(4):
                            nc.tensor.matmul(
                                ps[:, :nb],
                                xts[k][:, bi * 128 : (bi + 1) * 128],
                                wts[k][:, nb0 : nb0 + nb],
                                start=(k == 0),
                                stop=(k == 3),
                            )
                        ex = epool.tile([128, NBLK], F32, tag="ex")
                        nc.scalar.activation(
                            ex[:, :nb],
                            ps[:, :nb],
                            FN.Exp,
                            bias=-M0,
                            scale=S,
                            accum_out=zbufs[bi][:, col : col + 1],
                        )
                    col += 1
                    nb0 += nb
                c0 += sw
            ncol = col  # 25

            # partial Z per core -> DRAM [B] in natural batch order
            zdram = dram.tile([B], F32)
            for bi in range(4):
                zs = zpool.tile([128, 1], F32, tag=f"zs{bi}", name=f"zs{bi}")
                nc.vector.reduce_sum(
                    zs[:], zbufs[bi][:, :ncol], axis=mybir.AxisListType.X
                )
                nc.sync.dma_start(zdram[bi * 128 : (bi + 1) * 128], zs[:])

            zred = dram.tile([B], F32)
            nc.gpsimd.collective_compute(
                "AllReduce",
                mybir.AluOpType.add,
                replica_groups=[list(range(NCORES))],
                ins=[zdram.opt()],
                outs=[zred.opt()],
            )

            zsum = spool.tile([1, B], F32, tag="zsum")
            nc.sync.dma_start(zsum[:], zred.rearrange("(a b) -> a b", a=1))

            # label correction + final scalar
            t1 = spool.tile([1, B], F32, tag="t1")
            nc.scalar.activation(t1[:], sl_sb[:], FN.Exp, bias=-M0, scale=1.0)
            t2 = spool.tile([1, B], F32, tag="t2")
            nc.scalar.activation(t2[:], su_sb[:], FN.Exp, bias=-M0, scale=1.0)
            zc = spool.tile([1, B], F32, tag="zc")
            nc.vector.tensor_sub(zc[:], zsum[:], t1[:])
            nc.vector.tensor_add(zc[:], zc[:], t2[:])
            lg = spool.tile([1, B], F32, tag="lg")
            nc.scalar.activation(lg[:], zc[:], FN.Ln)
            v = spool.tile([1, B], F32, tag="v")
            nc.vector.tensor_sub(v[:], lg[:], su_sb[:])
            r = spool.tile([1, 1], F32, tag="r")
            nc.vector.reduce_sum(r[:], v[:], axis=mybir.AxisListType.X)
            ov = spool.tile([1, 1], F32, tag="ov")
            nc.scalar.activation(ov[:], r[:], FN.Copy, bias=M0, scale=1.0 / B)
            nc.sync.dma_start(out[:], ov[:])

    if not nc.is_finalized():
        nc.finalize()
    return nc


def _host_prep(input, label, weight):
    x = np.asarray(input, dtype=np.float32)
    lab = np.asarray(label).astype(np.int64).ravel()
    w = np.asarray(weight, dtype=np.float32)

    xn64 = x.astype(np.float64)
    xn64 /= np.maximum(
        np.sqrt(np.einsum("bd,bd->b", xn64, xn64))[:, None], 1e-12
    )
    xt = np.ascontiguousarray(xn64.T.astype(np.float32))  # [D, B]

    wn_inv = 1.0 / np.maximum(
        np.sqrt(np.einsum("cd,cd->c", w, w, dtype=np.float64)), 1e-12
    )
    wn = w * wn_inv[:, None].astype(np.float32)  # [C, D] normalized rows, f32

    # label terms (tiny, f64)
    wl = wn[lab].astype(np.float64)  # [B, D]
    cosl = np.einsum("bd,bd->b", xn64, wl)
    cosl = np.clip(cosl, -1.0, 1.0)
    sine = np.sqrt(np.maximum(1.0 - cosl * cosl, 0.0))
    phi = cosl * COS_M - sine * SIN_M
    phi = np.where(cosl > TH, phi, cosl - MM)
    sl = (S * cosl).astype(np.float32).reshape(1, B)
    su = (S * phi).astype(np.float32).reshape(1, B)

    # class-sharded, transposed W: [D, CS] per core, zero-padded at the tail
    shards = []
    for i in range(NCORES):
        lo, hi = i * CS, min((i + 1) * CS, C)
        sh = np.zeros((D, CS), dtype=np.float32)
        sh[:, : hi - lo] = wn[lo:hi].T
        shards.append(np.ascontiguousarray(sh))
    return xt, sl, su, shards


def kernel(input, label, weight):
    global _GRAPH, LAST_RESULT
    xt, sl, su, shards = _host_prep(input, label, weight)
    if _GRAPH is None:
        _GRAPH = _build_nc()
    in_maps = [
        {"xt": xt, "wt": shards[i], "sl": sl, "su": su} for i in range(NCORES)
    ]
    res = run_bass_kernel_spmd(_GRAPH, in_maps, list(range(NCORES)))
    LAST_RESULT = res
    outv = np.asarray(res.results[0]["out"], dtype=np.float32)
    return outv.reshape(())
